# revision 1
# baseline (speedup 1.0000x reference)
"""MoE top-2 routing kernel for Trainium2 (8 NeuronCores, expert-parallel).

Problem: nn_CoPRIMEModel_21861383537419 (moe_routing).
  x: (4, 2048, 1024), gate_W: (8, 1024), W1: (8, 1024, 4096), b1: (8, 4096),
  W2: (8, 4096, 1024), b2: (8, 1024).  Top-2 of 8 experts, exact GELU.

Strategy (expert parallel, per the sharding hint): the host computes gate
logits only to DECIDE token placement (top-2 expert indices), gathers each
expert's tokens, and ships expert e's weights + its gathered tokens to core e.
Each core recomputes the gate logits for its tokens ON DEVICE, derives its
own combine weight w = sigmoid(l_own - max_{e' != own} l_e') (exactly the
normalized top-2 softmax weight), runs the dense expert MLP
  y = w * (gelu(x @ W1 + b1) @ W2 + b2)
on the TensorEngine (bf16 operands, fp32 PSUM accumulation), and returns y^T.
The host scatter-adds the two pre-scaled per-expert contributions back into
token order.

Device layouts (tokens always the matmul free dim):
  mm1: psum[m,s] += W1[d,m]-as-lhsT . xgT[d,s]   (stationary = W1 tile)
  mm2: psum[d,s] += W2[m,d]-as-lhsT . hT[m,s]    (stationary = W2 tile)
"""

import os
from contextlib import ExitStack

import ml_dtypes
import numpy as np

import concourse.bacc as bacc
import concourse.bass_isa as bass_isa
import concourse.mybir as mybir
import concourse.tile as tile

B, S, D, M, E = 4, 2048, 1024, 4096, 8
P = 128
DT = D // P   # 8 d-tiles
MT = M // P   # 32 m-tiles
F32 = mybir.dt.float32

USE_F32 = bool(int(os.environ.get("MOE_F32", "0")))
CDT = F32 if USE_F32 else mybir.dt.bfloat16
NP_CDT = np.float32 if USE_F32 else ml_dtypes.bfloat16
MAXG = 896 if USE_F32 else 1024  # token group size (hT resident per group)

NEG = -1.0e30


def _groups(cap):
    """Split cap tokens into groups of <=MAXG, each into <=2 chunks of <=512."""
    gs, off = [], 0
    while off < cap:
        g = min(MAXG, cap - off)
        gs.append((off, g))
        off += g
    return gs


def _chunks(g):
    return [(0, 512), (512, g - 512)] if g > 512 else [(0, g)]


def build_nc(cap):
    """Build (and bacc-compile) the per-core SPMD expert kernel for capacity cap."""
    assert cap % P == 0
    nc = bacc.Bacc(
        "TRN2",
        target_bir_lowering=False,
        debug=False,
        enable_asserts=False,
        num_devices=1,
    )
    xgT = nc.dram_tensor("xgT", [D, cap], CDT, kind="ExternalInput").ap()
    w1 = nc.dram_tensor("w1", [D, M], CDT, kind="ExternalInput").ap()
    w2 = nc.dram_tensor("w2", [M, D], CDT, kind="ExternalInput").ap()
    b1v = nc.dram_tensor("b1v", [M], F32, kind="ExternalInput").ap()
    b2v = nc.dram_tensor("b2v", [D], F32, kind="ExternalInput").ap()
    gT = nc.dram_tensor("gT", [D, E], CDT, kind="ExternalInput").ap()
    esel = nc.dram_tensor("esel", [E, 1], F32, kind="ExternalInput").ap()
    negm = nc.dram_tensor("negm", [E, 1], F32, kind="ExternalInput").ap()
    yT = nc.dram_tensor("yT", [D, cap], F32, kind="ExternalOutput").ap()

    AF = mybir.ActivationFunctionType
    ALU = mybir.AluOpType

    with tile.TileContext(nc) as tc, ExitStack() as ctx:
        const = ctx.enter_context(tc.tile_pool(name="const", bufs=1))
        xg_pool = ctx.enter_context(tc.tile_pool(name="xg", bufs=DT))
        h_pool = ctx.enter_context(tc.tile_pool(name="h", bufs=MT))
        w_pool = ctx.enter_context(tc.tile_pool(name="w", bufs=3))
        y_pool = ctx.enter_context(tc.tile_pool(name="y", bufs=3))
        wb_pool = ctx.enter_context(tc.tile_pool(name="wb", bufs=2))
        rt_pool = ctx.enter_context(tc.tile_pool(name="rt", bufs=1))
        ps1 = ctx.enter_context(tc.tile_pool(name="ps1", bufs=3, space="PSUM"))
        ps2 = ctx.enter_context(tc.tile_pool(name="ps2", bufs=4, space="PSUM"))
        psr = ctx.enter_context(tc.tile_pool(name="psr", bufs=1, space="PSUM"))

        # --- constants ---
        gt_sb = const.tile([P, DT * E], CDT, tag="gt")
        for dt in range(DT):
            nc.sync.dma_start(
                gt_sb[:, dt * E : (dt + 1) * E], gT[dt * P : (dt + 1) * P, :]
            )
        b1_sb = const.tile([P, MT], F32, tag="b1")
        nc.sync.dma_start(b1_sb[:], b1v.rearrange("(t p) -> p t", p=P))
        b2_sb = const.tile([P, DT], F32, tag="b2")
        nc.sync.dma_start(b2_sb[:], b2v.rearrange("(t p) -> p t", p=P))
        esel_sb = const.tile([E, 1], F32, tag="esel")
        nc.sync.dma_start(esel_sb[:], esel[:])
        negm_sb = const.tile([E, 1], F32, tag="negm")
        nc.sync.dma_start(negm_sb[:], negm[:])
        ones_sb = const.tile([1, P], F32, tag="ones")
        nc.vector.memset(ones_sb[:], 1.0)

        for g0, G in _groups(cap):
            chunks = _chunks(G)

            # --- load this group's tokens (xgT columns g0:g0+G) ---
            xg = []
            for dt in range(DT):
                t = xg_pool.tile([P, MAXG], CDT, tag="xg")
                nc.sync.dma_start(
                    t[:, :G], xgT[dt * P : (dt + 1) * P, g0 : g0 + G]
                )
                xg.append(t)

            # --- routing: w = sigmoid(l_own - max_{e'!=own} l_e') per token ---
            w_bcast = wb_pool.tile([P, MAXG], F32, tag="wb")
            for c0, cw in chunks:
                lg_ps = psr.tile([E, 512], F32, tag="psr")
                for dt in range(DT):
                    nc.tensor.matmul(
                        lg_ps[:, :cw],
                        gt_sb[:, dt * E : (dt + 1) * E],
                        xg[dt][:, c0 : c0 + cw],
                        start=(dt == 0),
                        stop=(dt == DT - 1),
                    )
                lg = rt_pool.tile([E, 512], F32, tag="lg")
                nc.scalar.copy(lg[:, :cw], lg_ps[:, :cw])
                own = rt_pool.tile([E, 512], F32, tag="own")
                nc.vector.tensor_scalar(
                    own[:, :cw], lg[:, :cw], esel_sb[:], None, op0=ALU.mult
                )
                oth = rt_pool.tile([E, 512], F32, tag="oth")
                nc.vector.tensor_scalar(
                    oth[:, :cw], lg[:, :cw], negm_sb[:], None, op0=ALU.add
                )
                lo = rt_pool.tile([E, 512], F32, tag="lo")
                nc.gpsimd.partition_all_reduce(
                    lo[:, :cw], own[:, :cw], E, bass_isa.ReduceOp.add
                )
                mo = rt_pool.tile([E, 512], F32, tag="mo")
                nc.gpsimd.partition_all_reduce(
                    mo[:, :cw], oth[:, :cw], E, bass_isa.ReduceOp.max
                )
                dl = rt_pool.tile([1, 512], F32, tag="dl")
                nc.vector.tensor_sub(dl[:, :cw], lo[0:1, :cw], mo[0:1, :cw])
                wr = rt_pool.tile([1, 512], F32, tag="wr")
                nc.scalar.activation(wr[:, :cw], dl[:, :cw], AF.Sigmoid)
                wb_ps = psr.tile([P, 512], F32, tag="psr")
                nc.tensor.matmul(
                    wb_ps[:, :cw], ones_sb[:], wr[:, :cw], start=True, stop=True
                )
                nc.vector.tensor_copy(w_bcast[:, c0 : c0 + cw], wb_ps[:, :cw])

            # --- mm1: hT[m,s] = gelu(sum_d W1[d,m]^T xg[d,s] + b1[m]) ---
            w1r = w1.rearrange("(dt p) m -> p dt m", p=P)
            h_tiles = []
            w1t = None
            for mt in range(MT):
                if mt % 2 == 0:
                    w1t = w_pool.tile([P, DT, 2 * P], CDT, tag="w1", bufs=3)
                    nc.sync.dma_start(
                        w1t[:], w1r[:, :, mt * P : (mt + 2) * P]
                    )
                hps = [
                    ps1.tile([P, 512], F32, tag="ps1", name=f"hps{ci}")
                    for ci in range(len(chunks))
                ]
                for dt in range(DT):
                    lhs = w1t[:, dt, (mt % 2) * P : (mt % 2 + 1) * P]
                    for ci, (c0, cw) in enumerate(chunks):
                        nc.tensor.matmul(
                            hps[ci][:, :cw],
                            lhs,
                            xg[dt][:, c0 : c0 + cw],
                            start=(dt == 0),
                            stop=(dt == DT - 1),
                        )
                ht = h_pool.tile([P, MAXG], CDT, tag="h")
                for ci, (c0, cw) in enumerate(chunks):
                    nc.scalar.activation(
                        ht[:, c0 : c0 + cw],
                        hps[ci][:, :cw],
                        AF.Gelu,
                        bias=b1_sb[:, mt : mt + 1],
                    )
                h_tiles.append(ht)

            # --- mm2: y[d,s] = (sum_m W2[m,d] hT[m,s] + b2[d]) * w ---
            w2r = w2.rearrange("(mtb p) d -> p mtb d", p=P)
            for dtp in range(DT // 2):
                w2t = None
                yps = {}
                for mt in range(MT):
                    if mt % 8 == 0:
                        w2t = w_pool.tile([P, 8, 2 * P], CDT, tag="w2", bufs=3)
                        nc.sync.dma_start(
                            w2t[:],
                            w2r[
                                :,
                                mt : mt + 8,
                                dtp * 2 * P : (dtp + 1) * 2 * P,
                            ],
                        )
                    for dj in range(2):
                        lhs = w2t[:, mt % 8, dj * P : (dj + 1) * P]
                        for ci, (c0, cw) in enumerate(chunks):
                            if mt == 0:
                                yps[(dj, ci)] = ps2.tile(
                                    [P, 512], F32, tag="ps2", name=f"yps{dj}_{ci}"
                                )
                            nc.tensor.matmul(
                                yps[(dj, ci)][:, :cw],
                                lhs,
                                h_tiles[mt][:, c0 : c0 + cw],
                                start=(mt == 0),
                                stop=(mt == MT - 1),
                            )
                for dj in range(2):
                    dt = dtp * 2 + dj
                    for ci, (c0, cw) in enumerate(chunks):
                        ye = y_pool.tile([P, 512], F32, tag="y")
                        nc.vector.tensor_scalar(
                            ye[:, :cw],
                            yps[(dj, ci)][:, :cw],
                            b2_sb[:, dt : dt + 1],
                            None,
                            op0=ALU.add,
                        )
                        nc.vector.tensor_mul(
                            ye[:, :cw], ye[:, :cw], w_bcast[:, c0 : c0 + cw]
                        )
                        nc.sync.dma_start(
                            yT[dt * P : (dt + 1) * P, g0 + c0 : g0 + c0 + cw],
                            ye[:, :cw],
                        )

    nc.compile()
    return nc


_nc_cache = {}


def _get_nc(cap):
    if cap not in _nc_cache:
        _nc_cache[cap] = build_nc(cap)
    return _nc_cache[cap]


def host_route(xf, gate_W):
    """Top-2 expert indices per token (data placement decision only)."""
    logits = xf @ gate_W.T.astype(np.float32)
    top2 = np.argpartition(logits, E - 2, axis=1)[:, E - 2 :]
    idx = [np.nonzero((top2 == e).any(axis=1))[0] for e in range(E)]
    return idx


def make_in_maps(xf, gate_W, W1, b1, W2, b2, idx, cap):
    gTc = np.ascontiguousarray(gate_W.T).astype(NP_CDT)
    in_maps = []
    for e in range(E):
        xg = np.zeros((D, cap), NP_CDT)
        ne = len(idx[e])
        xg[:, :ne] = xf[idx[e]].T.astype(NP_CDT)
        esel_e = np.zeros((E, 1), np.float32)
        esel_e[e, 0] = 1.0
        negm_e = np.zeros((E, 1), np.float32)
        negm_e[e, 0] = NEG
        in_maps.append(
            {
                "xgT": xg,
                "w1": np.ascontiguousarray(W1[e]).astype(NP_CDT),
                "w2": np.ascontiguousarray(W2[e]).astype(NP_CDT),
                "b1v": np.ascontiguousarray(b1[e]),
                "b2v": np.ascontiguousarray(b2[e]),
                "gT": gTc,
                "esel": esel_e,
                "negm": negm_e,
            }
        )
    return in_maps


def kernel(**inputs):
    from concourse.bass_utils import run_bass_kernel_spmd

    x = np.asarray(inputs["x"], dtype=np.float32)
    gate_W = np.asarray(inputs["gate_W"], dtype=np.float32)
    W1 = np.asarray(inputs["W1"], dtype=np.float32)
    b1 = np.asarray(inputs["b1"], dtype=np.float32)
    W2 = np.asarray(inputs["W2"], dtype=np.float32)
    b2 = np.asarray(inputs["b2"], dtype=np.float32)

    Bs, Ss, Ds = x.shape
    xf = np.ascontiguousarray(x.reshape(-1, Ds))
    idx = host_route(xf, gate_W)
    cap = max(P, -(-max(len(i) for i in idx) // P) * P)

    nc = _get_nc(cap)
    in_maps = make_in_maps(xf, gate_W, W1, b1, W2, b2, idx, cap)
    res = run_bass_kernel_spmd(nc, in_maps, core_ids=list(range(E)))

    out = np.zeros_like(xf)
    for e in range(E):
        yTe = res.results[e]["yT"]  # [D, cap]
        ne = len(idx[e])
        out[idx[e]] += yTe[:, :ne].T
    return out.reshape(Bs, Ss, Ds)



# revision 2
# speedup vs baseline: 1.3600x; 1.3600x over previous
"""MoE top-2 routing kernel for Trainium2 (8 NeuronCores, expert-parallel).

Problem: nn_CoPRIMEModel_21861383537419 (moe_routing).
  x: (4, 2048, 1024), gate_W: (8, 1024), W1: (8, 1024, 4096), b1: (8, 4096),
  W2: (8, 4096, 1024), b2: (8, 1024).  Top-2 of 8 experts, exact GELU.

Strategy: expert parallel.  The host computes the gate (fp32), picks each
token's top-2 experts and exact combine weights, gathers each expert's
tokens, and ships them (fp8 e4m3, value + residual) to that expert's core.
Each core runs the dense expert MLP with error-compensated fp8 DoubleRow
matmuls (2 fp8 rows per PE pass = 4x bf16 matmul throughput; 3 of the 4
hi/res cross products are kept, so quantization error stays ~2e-3):

  mm1: psum = xh@W1h + xr@W1h + xh@W1r          (DoubleRow fp8, fp32 psum)
  h   = gelu(psum/32 + b1)  -> bf16 -> split into h_hi (fp8) + h_res (fp8)
  mm2: psum = hh@W2h + hr@W2h + hh@W2r
  y   = psum + 64*b2                              (bf16 out)

The host scatter-adds w/64 * y back into token order.  W1/W2 are scaled by
32/64 on the host so fp8 operands are ~N(0,1); the 1/32 is folded into the
gelu activation scale and the 1/64 into the host-side combine weight.

Device layouts (tokens always the matmul moving dim):
  mm1: psum[m, s] += W1[d, m]-as-lhsT . xT[d, s]    (128x128 stationary)
  mm2: psum[d, s] += W2[m, d]-as-lhsT . hT[m, s]
W2 (hi+res) stays SBUF-resident; x and W1 tiles stream per column group.
h_hi/h_res are double-buffered by group parity.
"""

from contextlib import ExitStack

import ml_dtypes
import numpy as np

import concourse.bacc as bacc
import concourse.mybir as mybir
import concourse.tile as tile

B, S, D, M, E = 4, 2048, 1024, 4096, 8
P = 128
DT = D // P    # 8 d-tiles
MT = M // P    # 32 m-tiles
KP1 = DT // 2  # 4 k-pairs in mm1
KP2 = MT // 2  # 16 k-pairs in mm2

F32 = mybir.dt.float32
BF16 = mybir.dt.bfloat16
FP8 = mybir.dt.float8e4
NPF8 = ml_dtypes.float8_e4m3
DRM = mybir.MatmulPerfMode.DoubleRow

S1 = 32.0   # host scale on W1 (folded out via gelu scale)
S2 = 64.0   # host scale on W2 (folded out via host combine weight)
GW_TARGET = 576  # max column-group width (h tiles are sized by this)


def _groups(cap):
    """Split cap into contiguous groups of 128-multiples, each <= ~GW."""
    nb = cap // P
    ng = max(1, -(-cap // GW_TARGET))
    base, extra = divmod(nb, ng)
    out, off = [], 0
    for g in range(ng):
        w = (base + (1 if g < extra else 0)) * P
        out.append((off, w))
        off += w
    return out


def _chunks(gw):
    out, off = [], 0
    while off < gw:
        w = min(256, gw - off)
        out.append((off, w))
        off += w
    return out


def build_nc(cap):
    assert cap % P == 0
    nc = bacc.Bacc(
        "TRN2",
        target_bir_lowering=False,
        debug=False,
        enable_asserts=False,
        num_devices=1,
    )
    xh = nc.dram_tensor("xh", [D, cap], FP8, kind="ExternalInput").ap()
    xr = nc.dram_tensor("xr", [D, cap], FP8, kind="ExternalInput").ap()
    # packed per m-tile: [mt][p][hi|res][dt*128+j]
    w1p = nc.dram_tensor("w1p", [MT, P, 2, DT * P], FP8,
                         kind="ExternalInput").ap()
    w2h = nc.dram_tensor("w2h", [M, D], FP8, kind="ExternalInput").ap()
    w2r = nc.dram_tensor("w2r", [M, D], FP8, kind="ExternalInput").ap()
    b1v = nc.dram_tensor("b1v", [M], F32, kind="ExternalInput").ap()
    b2v = nc.dram_tensor("b2v", [D], F32, kind="ExternalInput").ap()  # 64*b2
    yT = nc.dram_tensor("yT", [D, cap], BF16, kind="ExternalOutput").ap()

    AF = mybir.ActivationFunctionType
    ALU = mybir.AluOpType
    groups = _groups(cap)
    gwmax = max(gw for _, gw in groups)

    with tile.TileContext(nc) as tc, ExitStack() as ctx:
        const = ctx.enter_context(tc.tile_pool(name="const", bufs=1))
        x_pool = ctx.enter_context(tc.tile_pool(name="xg", bufs=2))
        w1_pool = ctx.enter_context(tc.tile_pool(name="w1", bufs=4))
        hm_pool = ctx.enter_context(tc.tile_pool(name="hm", bufs=4))
        y_pool = ctx.enter_context(tc.tile_pool(name="y", bufs=4))
        ps1 = ctx.enter_context(tc.tile_pool(name="ps1", bufs=3, space="PSUM"))
        ps2 = ctx.enter_context(tc.tile_pool(name="ps2", bufs=3, space="PSUM"))

        # --- biases ---
        b1_sb = const.tile([P, MT], F32, tag="b1")
        nc.sync.dma_start(b1_sb[:], b1v.rearrange("(t p) -> p t", p=P))
        b2_sb = const.tile([P, DT], F32, tag="b2")
        nc.sync.dma_start(b2_sb[:], b2v.rearrange("(t p) -> p t", p=P))

        # --- resident W2 (hi + res), loaded once on the ACT queue ---
        w2h_sb = const.tile([P, MT, D], FP8, tag="w2h")
        w2r_sb = const.tile([P, MT, D], FP8, tag="w2r")
        w2h_r = w2h.rearrange("(t p) d -> p t d", p=P)
        w2r_r = w2r.rearrange("(t p) d -> p t d", p=P)
        for j in range(4):
            sl = slice(j * (MT // 4), (j + 1) * (MT // 4))
            nc.scalar.dma_start(w2h_sb[:, sl, :], w2h_r[:, sl, :])
            nc.scalar.dma_start(w2r_sb[:, sl, :], w2r_r[:, sl, :])

        # --- h buffers, double-buffered by group parity ---
        hbuf = []
        for par in range(2):
            hhi = const.tile([P, MT, gwmax], FP8, tag=f"hhi{par}")
            hres = const.tile([P, MT, gwmax], FP8, tag=f"hres{par}")
            hbuf.append((hhi, hres))

        xh_r = xh.rearrange("(t p) c -> p t c", p=P)
        xr_r = xr.rearrange("(t p) c -> p t c", p=P)

        for gi, (g0, gw) in enumerate(groups):
            chunks = _chunks(gw)
            hhi, hres = hbuf[gi % 2]

            # --- this group's tokens (value + residual) ---
            xh_sb = x_pool.tile([P, DT, gwmax], FP8, tag="xh")
            nc.sync.dma_start(xh_sb[:, :, :gw], xh_r[:, :, g0:g0 + gw])
            xr_sb = x_pool.tile([P, DT, gwmax], FP8, tag="xr")
            nc.sync.dma_start(xr_sb[:, :, :gw], xr_r[:, :, g0:g0 + gw])

            # --- mm1 + gelu + h split, m-tile major (W1 streams per mt) ---
            for mt in range(MT):
                w1t = w1_pool.tile([P, 2, DT, P], FP8, tag="w1")
                nc.sync.dma_start(w1t[:], w1p[mt])
                for c0, cw in chunks:
                    ps = ps1.tile([P, 512], F32, tag="ps1")
                    mm = 0
                    for hr, xt in ((0, xh_sb), (0, xr_sb), (1, xh_sb)):
                        for dp in range(KP1):
                            nc.tensor.matmul(
                                ps[:, :cw],
                                w1t[:, hr, 2 * dp:2 * dp + 2, :],
                                xt[:, 2 * dp:2 * dp + 2, c0:c0 + cw],
                                start=(mm == 0),
                                stop=(mm == 3 * KP1 - 1),
                                perf_mode=DRM,
                            )
                            mm += 1
                    hm = hm_pool.tile([P, 256], BF16, tag="hm")
                    nc.scalar.activation(
                        hm[:, :cw], ps[:, :cw], AF.Gelu,
                        bias=b1_sb[:, mt:mt + 1], scale=1.0 / S1,
                    )
                    nc.vector.tensor_copy(hhi[:, mt, c0:c0 + cw], hm[:, :cw])
                    nc.gpsimd.tensor_sub(
                        hres[:, mt, c0:c0 + cw], hm[:, :cw],
                        hhi[:, mt, c0:c0 + cw],
                    )

            # --- mm2 + bias + store ---
            for c0, cw in chunks:
                for dt in range(DT):
                    ps = ps2.tile([P, 512], F32, tag="ps2")
                    mm = 0
                    for p2 in range(KP2):
                        for wsb, hsb in ((w2h_sb, hhi), (w2h_sb, hres),
                                         (w2r_sb, hhi)):
                            nc.tensor.matmul(
                                ps[:, :cw],
                                wsb[:, 2 * p2:2 * p2 + 2,
                                    dt * P:(dt + 1) * P],
                                hsb[:, 2 * p2:2 * p2 + 2, c0:c0 + cw],
                                start=(mm == 0),
                                stop=(mm == 3 * KP2 - 1),
                                perf_mode=DRM,
                            )
                            mm += 1
                    ye = y_pool.tile([P, 256], BF16, tag="y")
                    nc.vector.tensor_scalar(
                        ye[:, :cw], ps[:, :cw], b2_sb[:, dt:dt + 1], None,
                        op0=ALU.add,
                    )
                    nc.scalar.dma_start(
                        yT[dt * P:(dt + 1) * P, g0 + c0:g0 + c0 + cw],
                        ye[:, :cw],
                    )

    nc.compile()
    return nc


_nc_cache = {}


def _get_nc(cap):
    if cap not in _nc_cache:
        _nc_cache[cap] = build_nc(cap)
    return _nc_cache[cap]


def host_route(xf, gate_W):
    """Exact fp32 top-2 routing: per-expert token indices + combine weights."""
    logits = xf @ gate_W.T.astype(np.float32)
    mx = logits.max(axis=1, keepdims=True)
    p = np.exp(logits - mx)
    p /= p.sum(axis=1, keepdims=True)
    top_i = np.argsort(-p, axis=1, kind="stable")[:, :2]
    top_w = np.take_along_axis(p, top_i, axis=1)
    top_w = top_w / (top_w.sum(axis=1, keepdims=True) + 1e-9)
    idx, wts = [], []
    for e in range(E):
        hit = top_i == e
        sel = hit.any(axis=1)
        ids = np.nonzero(sel)[0]
        w = np.where(hit[ids, 0], top_w[ids, 0], top_w[ids, 1])
        idx.append(ids)
        wts.append(w.astype(np.float32))
    return idx, wts


def _q8(a):
    return a.astype(NPF8)


def _pack_w1(w1h, w1r):
    """[D, M] hi/res -> [MT, P, 2, DT*128] so each m-tile loads in one DMA."""
    def lay(a):
        return (a.reshape(DT, P, MT, P).transpose(2, 1, 0, 3)
                .reshape(MT, P, DT * P))
    return np.ascontiguousarray(
        np.stack([lay(w1h), lay(w1r)], axis=2))


def make_in_maps(xf, W1, b1, W2, b2, idx, cap):
    xq = _q8(xf)
    xrq = _q8(xf - xq.astype(np.float32))
    in_maps = []
    for e in range(E):
        ne = len(idx[e])
        xhT = np.zeros((D, cap), NPF8)
        xhT[:, :ne] = xq[idx[e]].T
        xrT = np.zeros((D, cap), NPF8)
        xrT[:, :ne] = xrq[idx[e]].T
        w1s = S1 * W1[e]
        w1h = _q8(w1s)
        w1r = _q8(w1s - w1h.astype(np.float32))
        w2s = S2 * W2[e]
        w2h = _q8(w2s)
        w2r = _q8(w2s - w2h.astype(np.float32))
        in_maps.append({
            "xh": xhT,
            "xr": xrT,
            "w1p": _pack_w1(w1h, w1r),
            "w2h": np.ascontiguousarray(w2h),
            "w2r": np.ascontiguousarray(w2r),
            "b1v": np.ascontiguousarray(b1[e]),
            "b2v": np.ascontiguousarray(S2 * b2[e]),
        })
    return in_maps


def kernel(**inputs):
    from concourse.bass_utils import run_bass_kernel_spmd

    x = np.asarray(inputs["x"], dtype=np.float32)
    gate_W = np.asarray(inputs["gate_W"], dtype=np.float32)
    W1 = np.asarray(inputs["W1"], dtype=np.float32)
    b1 = np.asarray(inputs["b1"], dtype=np.float32)
    W2 = np.asarray(inputs["W2"], dtype=np.float32)
    b2 = np.asarray(inputs["b2"], dtype=np.float32)

    Bs, Ss, Ds = x.shape
    xf = np.ascontiguousarray(x.reshape(-1, Ds))
    idx, wts = host_route(xf, gate_W)
    cap = max(P, -(-max(len(i) for i in idx) // P) * P)

    nc = _get_nc(cap)
    in_maps = make_in_maps(xf, W1, b1, W2, b2, idx, cap)
    res = run_bass_kernel_spmd(nc, in_maps, core_ids=list(range(E)))

    out = np.zeros_like(xf)
    for e in range(E):
        yTe = res.results[e]["yT"]  # [D, cap] bf16, scaled by S2
        ne = len(idx[e])
        out[idx[e]] += (wts[e] / S2)[:, None] * \
            yTe[:, :ne].T.astype(np.float32)
    return out.reshape(Bs, Ss, Ds)
